# revision 12
# baseline (speedup 1.0000x reference)
"""GRU cell on 8 Trainium2 NeuronCores.

Reference computation (B=65536, D=256):
    z = sigmoid(x@Wz + h@Uz + bz)
    r = sigmoid(x@Wr + h@Ur + br)
    h_hat = tanh(x@Wh + (r*h)@Uh + bh)
    h_t = z*h + (1-z)*h_hat  ; returns (h_t, h_t)

Strategy: data-parallel over the batch dim (8 shards of 8192 rows), all
fp16 on chip (rel_l2 ~1.1e-3 vs the f32 reference; gate is 2e-2).  The
matmul stream runs at the fp16 PE issue floor (24 matmul passes over
8192 batch cols = 196608 PE cycles = 81.9us at 2.4GHz), so everything
else is about the head (framework preamble 7.2us + first-DMA landing)
and the tail (last ACT/DVE chain + final store receipt + postamble).
Key structure vs the 104.4us version:
  * head DMAs split fine (12x65KB weight pieces + 256-col x/h tiles)
    and spread over sync/scalar/vector/gpsimd trigger queues in
    need-order: HWDGE rings are FIFO per issuing engine, so the
    critical first pieces are never queued behind bulk bytes.  8 HWDGE
    + 8 SWDGE completion-sem lanes bound the immediate-trigger budget.
  * the first two work items are 256 cols wide so the first real
    matmuls need only 65KB tiles; the last 512 cols are split
    256+128+128 so the serial ACT+DVE+store tail after the final
    matmul is short.
  * both activation tables are force-loaded early (dummy sigmoid+tanh
    on the warmup tile) - otherwise the tanh table load (1.28us) sits
    behind the scalar DMA triggers and stalls the first r-sigmoid,
    which showed up as candidate-matmul stalls in the trace.
  * warmup matmuls (N=256, cold 213ns each) hold the PE busy from
    ~7.5us so the HAM clock gate lifts 1.2->2.4GHz right as the first
    input tiles land (~10us); sized to end at data-arrival.
  * tail pieces use the short combine: u=sigmoid(-a_z) on ACT (bias
    -bz), m1=z*h early on DVE; after tanh only v=u*hh, o=v+m1 remain.
    Tail stores go 2-way on the idle sync/scalar HWDGE queues.
  * r-gate of item i+1 is computed one iteration early so its sigmoid
    + r*h never gate the candidate matmuls.
"""

import os
import sys

for _p in ("/opt/trn_rl_repo", "/root/.axon_site/_ro/trn_rl_repo"):
    if os.path.isdir(_p) and _p not in sys.path:
        sys.path.append(_p)

import numpy as np

B = 65536
D = 256
N_CORES = 8
S = B // N_CORES  # batch rows per core

# Input-tile load plan.
# Narrow per-block tiles for the first 512 cols (fast head fill), packed
# 4-block tiles for the bulk (DMA efficiency: 1KB+ per-partition lines).
NARROW = [(0, 256), (256, 256)]
PACKED = [(512, 512), (1024, 512), (1536, 512)] + [
    (2048 + 1024 * i, 1024) for i in range(6)
]
_BLOCKS = ("x0", "x1", "h0", "h1")
# matrix order inside the packed weight tensor
_WORDER = ("Wr", "Ur", "Wz", "Uz", "Wh", "Uh")

# Work items: (dram col start, width, psum col offset).  First two are
# narrow (head fill), last three shrink so the post-stream tail is short.
WI = (
    [(0, 256, 0), (256, 256, 256)]
    + [(512 + 512 * i, 512, 0) for i in range(14)]
    + [(7680, 256, 0), (7936, 256, 256)]
)


def build_nc(s=S, mm_dtype_name=None):
    """Build + compile the per-core Bass program for a shard of s rows."""
    import concourse.bass as bass
    import concourse.mybir as mybir
    import concourse.tile as tile
    from concourse import bacc

    f32 = mybir.dt.float32
    if mm_dtype_name is None:
        mm_dtype_name = os.environ.get("GRU_MM_DTYPE", "float16")
    f16 = getattr(mybir.dt, mm_dtype_name)
    AF = mybir.ActivationFunctionType
    n_warm = int(os.environ.get("GRU_WARMUP", "16"))

    nc = bacc.Bacc("TRN2", target_bir_lowering=False)
    xh = nc.dram_tensor("xh", [128, 4, s], f16, kind="ExternalInput")
    wcat = nc.dram_tensor("wcat", [D, 6 * D], f16, kind="ExternalInput")
    bcat = nc.dram_tensor("bcat", [128, 8], f32, kind="ExternalInput")
    outT = nc.dram_tensor("outT", [D, s], f16, kind="ExternalOutput")

    nwi = len(WI)

    with tile.TileContext(nc) as tc:
        with (
            tc.tile_pool(name="const", bufs=1) as cpool,
            tc.tile_pool(name="work", bufs=2) as wpool,
            tc.tile_pool(name="outb", bufs=4) as opool,
            tc.tile_pool(name="psum", bufs=1, space=bass.MemorySpace.PSUM) as ppool,
        ):
            # ---- PE warmup -------------------------------------------------
            # The HAM clock gate needs ~3.4us of sustained PE activity to
            # lift the engine from 1.2 to 2.4 GHz; the PE is idle during the
            # head DMAs anyway, so burn that window on dummy matmuls (cold
            # N=256 MM = 213ns each), sized to end when the input tiles land.
            zt0 = cpool.tile([128, 256], f16, tag="warm", name="warm")
            nc.gpsimd.memset(zt0[:], 0)
            pw = ppool.tile([128, 256], f32, tag="pwarm", name="pwarm")
            for _ in range(n_warm):
                nc.tensor.matmul(pw[:], zt0[:, 0:128], zt0[:], start=True, stop=True)
            wsink = cpool.tile([128, 256], f32, tag="wsink", name="wsink")
            nc.vector.tensor_copy(wsink[:], pw[:])

            # ---- head DMA schedule ----------------------------------------
            # Only sync/scalar (HWDGE) + gpsimd (SWDGE) can trigger DMAs.
            # HWDGE rings are FIFO per issuing engine and there are 8 HWDGE
            # + 8 SWDGE completion-sem lanes; each trigger instruction also
            # occupies its queue ~0.6us.  So: critical pieces first on their
            # rings in need-order, per-gate weight chunks (131KB) to keep
            # the trigger count affordable.
            wchunk = {}  # (gate 0..2, k) -> [128, 512] = W|U for that gate
            for c in range(3):
                for k in range(2):
                    wchunk[(c, k)] = cpool.tile(
                        [128, 512], f16, tag=f"w{c}{k}", name=f"w{c}{k}"
                    )

            def wload(c, k, eng):
                eng.dma_start(
                    wchunk[(c, k)][:],
                    wcat[k * 128 : (k + 1) * 128, c * 512 : (c + 1) * 512],
                )

            inp = {}  # (block, ('n'|'p', idx)) -> AP [128, width]

            def load_narrow(blk, li, eng):
                bi = _BLOCKS.index(blk)
                start, width = NARROW[li]
                t = cpool.tile([128, width], f16, tag=f"i{blk}_{li}",
                               name=f"i{blk}_{li}")
                eng.dma_start(t[:], xh[:, bi, start : start + width])
                inp[(blk, ("n", li))] = t[:]

            # sync: first x half, then k=0 weight chunks in gate order
            # (w00 must be 2nd on its ring: per-queue HBM bandwidth is
            # roughly fair-shared, so a chunk 3rd-in-line lands ~13us)
            load_narrow("x0", 0, nc.sync)
            wload(0, 0, nc.sync)
            wload(1, 0, nc.sync)
            wload(2, 0, nc.sync)
            # scalar: second x half, then k=1 weight chunks
            load_narrow("x1", 0, nc.scalar)
            wload(0, 1, nc.scalar)
            wload(1, 1, nc.scalar)
            wload(2, 1, nc.scalar)
            def load_packed(li, eng):
                start, width = PACKED[li]
                t = cpool.tile([128, 4, width], f16, tag=f"ixh_{li}",
                               name=f"ixh_{li}")
                eng.dma_start(t[:], xh[:, :, start : start + width])
                for bi, blk in enumerate(_BLOCKS):
                    inp[(blk, ("p", li))] = t[:, bi, :]

            # gpsimd (SWDGE, own 8 lanes): j=0 h tiles, bias, j=1 tiles
            load_narrow("h0", 0, nc.gpsimd)
            load_narrow("h1", 0, nc.gpsimd)
            b_sb = cpool.tile([128, 8], f32, tag="bcat")
            nc.gpsimd.dma_start(b_sb[:], bcat[:, :])
            load_narrow("x0", 1, nc.gpsimd)
            load_narrow("x1", 1, nc.gpsimd)
            load_narrow("h0", 1, nc.gpsimd)
            load_narrow("h1", 1, nc.gpsimd)
            # force both ACT tables (sigmoid + tanh) to load now: lazily
            # the tanh load would sit mid-queue and stall the first sigmoid
            dume = cpool.tile([128, 1], f16, tag="dume", name="dume")
            nc.scalar.activation(dume[:], zt0[:, 0:1], AF.Sigmoid)
            nc.scalar.activation(dume[:], zt0[:, 0:1], AF.Tanh)
            # bulk input stream on sync's rotation-paced lanes, behind the
            # weight chunks on the same ring so it can't steal their BW
            for li in range(len(PACKED)):
                load_packed(li, nc.sync)

            def wap(i, k, g):
                """Weight AP [128,128]: matrix i (order _WORDER),
                contraction half k, output-feature half g."""
                off = (i % 2) * 256 + g * 128
                return wchunk[(i // 2, k)][:, off : off + 128]

            def inp_ap(blk, c0, w):
                for li, (start, width) in enumerate(NARROW):
                    if start <= c0 and c0 + w <= start + width:
                        return inp[(blk, ("n", li))][:, c0 - start : c0 - start + w]
                for li, (start, width) in enumerate(PACKED):
                    if start <= c0 and c0 + w <= start + width:
                        return inp[(blk, ("p", li))][:, c0 - start : c0 - start + w]
                raise ValueError((blk, c0, w))

            def operands(i):
                c0, w, _ = WI[i]
                xs = [inp_ap(f"x{k}", c0, w) for k in range(2)]
                hs = [inp_ap(f"h{k}", c0, w) for k in range(2)]
                return xs, hs

            def gate_pair(tagbase, wi, ui, xs, rhs_u, po, w):
                """Both g-halves of one gate.  W (x-side) matmuls of both
                halves run before the U matmuls: x tiles arrive from HBM
                before h tiles, and for the candidate gate this gives the
                r*h producer extra slack.  k-major within each pass."""
                ps = []
                for g in range(2):
                    p = ppool.tile([128, 512], f32, tag=f"{tagbase}{g}",
                                   name=f"{tagbase}{g}")
                    ps.append(p)
                for k in range(2):
                    for g in range(2):
                        nc.tensor.matmul(ps[g][:, po : po + w], wap(wi, k, g),
                                         xs[k], start=(k == 0), stop=False)
                for k in range(2):
                    for g in range(2):
                        nc.tensor.matmul(ps[g][:, po : po + w], wap(ui, k, g),
                                         rhs_u[k], start=False, stop=(k == 1))
                return ps

            def r_gate(i):
                """reset gate -> r*h tiles for work item i."""
                c0, w, po = WI[i]
                xs, hs = operands(i)
                ps = gate_pair("pr", 0, 1, xs, hs, po, w)
                rh = []
                for g in range(2):
                    rt = wpool.tile([128, 512], f16, tag=f"r{g}", name=f"r{g}")
                    nc.scalar.activation(rt[:, 0:w], ps[g][:, po : po + w],
                                         AF.Sigmoid, bias=b_sb[:, g : g + 1])
                    t = wpool.tile([128, 512], f16, tag=f"rh{g}", name=f"rh{g}")
                    nc.vector.tensor_mul(t[:, 0:w], rt[:, 0:w], hs[g])
                    rh.append(t[:, 0:w])
                return rh

            def combine_short(g, hs, ut, m1, hh, w):
                """o = z*h + (1-z)*hh = m1 - u*hh with u=z-1, m1=z*h
                precomputed: only two DVE links after the tanh."""
                v = wpool.tile([128, 512], f16, tag=f"v{g}", name=f"v{g}")
                nc.vector.tensor_mul(v[:, 0:w], ut[g][:, 0:w], hh[:, 0:w])
                o = opool.tile([128, 512], f16, tag=f"o{g}", name=f"o{g}")
                nc.vector.tensor_sub(o[:, 0:w], m1[g][:, 0:w], v[:, 0:w])
                return o

            def zu_gate(i, emit_um):
                """z-sigmoids for item i; with emit_um also u=z-1, m1=z*h."""
                c0, w, po = WI[i]
                xs, hs = operands(i)
                pz = gate_pair("pz", 2, 3, xs, hs, po, w)
                zt, ut, m1 = [], [], []
                for g in range(2):
                    t = wpool.tile([128, 512], f16, tag=f"z{g}", name=f"z{g}")
                    nc.scalar.activation(t[:, 0:w], pz[g][:, po : po + w],
                                         AF.Sigmoid, bias=b_sb[:, 2 + g : 3 + g])
                    zt.append(t)
                    if emit_um:
                        u = wpool.tile([128, 512], f16, tag=f"u{g}", name=f"u{g}")
                        nc.vector.tensor_scalar_sub(u[:, 0:w], t[:, 0:w], 1.0)
                        ut.append(u)
                        m = wpool.tile([128, 512], f16, tag=f"zh{g}", name=f"zh{g}")
                        nc.vector.tensor_mul(m[:, 0:w], t[:, 0:w], hs[g])
                        m1.append(m)
                return zt, ut, m1

            # software pipeline: r-gate one work item ahead of z/candidate.
            rh_cur = r_gate(0)
            for i in range(nwi - 1):
                c0, w, po = WI[i]
                xs, hs = operands(i)
                if i == 0:
                    rh_next = None
                elif i == 1:
                    rh_cur = r_gate(1)
                    rh_next = r_gate(2) if nwi > 2 else None
                else:
                    rh_next = r_gate(i + 1) if i + 1 < nwi else None

                tail = i == nwi - 2
                zt, ut, m1 = zu_gate(i, tail)
                ph = gate_pair("ph", 4, 5, xs, rh_cur, po, w)
                for g in range(2):
                    hh = wpool.tile([128, 512], f16, tag=f"hh{g}", name=f"hh{g}")
                    nc.scalar.activation(hh[:, 0:w], ph[g][:, po : po + w],
                                         AF.Tanh, bias=b_sb[:, 4 + g : 5 + g])
                    if tail:
                        o = combine_short(g, hs, ut, m1, hh, w)
                    else:
                        o = opool.tile([128, 512], f16, tag=f"o{g}", name=f"o{g}")
                        d = wpool.tile([128, 512], f16, tag=f"d{g}", name=f"d{g}")
                        nc.vector.tensor_sub(d[:, 0:w], hs[g], hh[:, 0:w])
                        m = wpool.tile([128, 512], f16, tag=f"m{g}", name=f"m{g}")
                        nc.vector.tensor_mul(m[:, 0:w], zt[g][:, 0:w], d[:, 0:w])
                        nc.vector.tensor_add(o[:, 0:w], hh[:, 0:w], m[:, 0:w])
                    orow = outT[g * 128 : (g + 1) * 128, :]
                    # bulk stores ride gpsimd's SWDGE lanes so they never
                    # contend with the HWDGE traffic.  The second-to-last
                    # item's stores go on sync (NOT scalar: a 0.65us store
                    # trigger between the final tanhs delays the drain).
                    eng = nc.sync if tail else nc.gpsimd
                    eng.dma_start(orow[:, c0 : c0 + w], o[:, 0:w])
                rh_cur = rh_next

            # Last item, drain-optimized: z first (sigmoid + u/m1 while
            # the candidate matmuls run), candidate g-split so after the
            # very last matmul only tanh(g1) -> v -> o -> trigger remain.
            i = nwi - 1
            c0, w, po = WI[i]
            xs, hs = operands(i)
            zt, ut, m1 = zu_gate(i, True)
            for g in range(2):
                p = ppool.tile([128, 512], f32, tag=f"ph{g}", name=f"ph{g}")
                for k in range(2):
                    nc.tensor.matmul(p[:, po : po + w], wap(4, k, g), xs[k],
                                     start=(k == 0), stop=False)
                for k in range(2):
                    nc.tensor.matmul(p[:, po : po + w], wap(5, k, g),
                                     rh_cur[k], start=False, stop=(k == 1))
                hh = wpool.tile([128, 512], f16, tag=f"hh{g}", name=f"hh{g}")
                nc.scalar.activation(hh[:, 0:w], p[:, po : po + w],
                                     AF.Tanh, bias=b_sb[:, 4 + g : 5 + g])
                o = combine_short(g, hs, ut, m1, hh, w)
                orow = outT[g * 128 : (g + 1) * 128, :]
                eng = nc.scalar if g == 1 else nc.sync
                eng.dma_start(orow[:, c0 : c0 + w], o[:, 0:w])

    nc.compile()
    return nc


_NC_CACHE = {}


def _get_nc():
    key = (S, os.environ.get("GRU_MM_DTYPE", "float16"),
           os.environ.get("GRU_WARMUP", "12"))
    if key not in _NC_CACHE:
        _NC_CACHE[key] = build_nc(S, key[1])
    return _NC_CACHE[key]


def _make_in_maps(inputs):
    f32 = np.float32
    dt16 = {"float16": np.float16}.get(
        os.environ.get("GRU_MM_DTYPE", "float16")
    )
    if dt16 is None:
        import ml_dtypes

        dt16 = ml_dtypes.bfloat16
    x = np.asarray(inputs["x"], f32)
    h = np.asarray(inputs["h_t_1"], f32)
    wcat = np.ascontiguousarray(
        np.concatenate(
            [np.asarray(inputs[n], f32) for n in ("Wr", "Ur", "Wz", "Uz", "Wh", "Uh")],
            axis=1,
        ).astype(dt16)
    )
    bz = np.asarray(inputs["bz"], f32)
    bcat = np.ascontiguousarray(
        np.concatenate(
            [
                np.asarray(inputs["br"], f32).reshape(2, 128).T,
                bz.reshape(2, 128).T,
                np.asarray(inputs["bh"], f32).reshape(2, 128).T,
                (-bz).reshape(2, 128).T,
            ],
            axis=1,
        )
    )
    consts = {"wcat": wcat, "bcat": bcat}
    in_maps = []
    for c in range(N_CORES):
        sl = slice(c * S, (c + 1) * S)
        xT = x[sl].T.astype(dt16)  # [256, S]
        hT = h[sl].T.astype(dt16)
        xhm = np.empty((128, 4, S), dt16)
        xhm[:, 0] = xT[0:128]
        xhm[:, 1] = xT[128:256]
        xhm[:, 2] = hT[0:128]
        xhm[:, 3] = hT[128:256]
        m = {"xh": np.ascontiguousarray(xhm)}
        m.update(consts)
        in_maps.append(m)
    return in_maps


def run(inputs, trace=False):
    """Run on hardware; returns (h_t ndarray, BassKernelResults)."""
    from concourse.bass_utils import run_bass_kernel_spmd

    nc = _get_nc()
    in_maps = _make_in_maps(inputs)
    res = run_bass_kernel_spmd(nc, in_maps, list(range(N_CORES)), trace=trace)
    out = np.empty((B, D), np.float32)
    for c in range(N_CORES):
        out[c * S : (c + 1) * S] = res.results[c]["outT"].T.astype(np.float32)
    return out, res


def kernel(**inputs):
    out, _ = run(inputs, trace=False)
    return (out, out)


# revision 13
# speedup vs baseline: 1.0114x; 1.0114x over previous
"""GRU cell on 8 Trainium2 NeuronCores.

Reference computation (B=65536, D=256):
    z = sigmoid(x@Wz + h@Uz + bz)
    r = sigmoid(x@Wr + h@Ur + br)
    h_hat = tanh(x@Wh + (r*h)@Uh + bh)
    h_t = z*h + (1-z)*h_hat  ; returns (h_t, h_t)

Strategy: data-parallel over the batch dim (8 shards of 8192 rows), all
fp16 on chip (rel_l2 ~1.1e-3 vs the f32 reference; gate is 2e-2).  The
matmul stream runs at the fp16 PE issue floor (24 matmul passes over
8192 batch cols = 196608 PE cycles = 81.9us at 2.4GHz), so the
remaining time is the fixed framework preamble (~7.2us), the initial
HBM fill, and the drain tail + postamble.  Key structure:
  * host packs each shard as [128 partitions, 4 blocks, 8192] fp16
    (blocks = x k0, x k1, h k0, h k1): the contraction dim of all six
    GEMMs is the SBUF partition dim, fp16 halves HBM traffic and
    streams the PE at full rate with fast weight load.
  * all input tiles are SBUF-resident (8.4MB of 24MB); tiles are 512+
    cols so per-partition DMA lines stay at/above the 1KB efficiency
    knee (256-col tiles fragment to 512B lines and halve effective DMA
    bandwidth - measured).  Head DMAs are spread over the sync/scalar
    HWDGE rings (FIFO per ring, ~8 completion-sem lanes total) and
    gpsimd's SWDGE ring in need-order; the bulk stream self-paces on
    sync's lane rotation.
  * both ACT tables (sigmoid+tanh) are force-loaded via dummy
    activations right after scalar's head triggers - lazily, the tanh
    table load (1.28us) sits mid-queue and stalls the first r-sigmoid
    (measured as candidate-matmul stalls).
  * dummy warm-up matmuls during the head DMAs hold the PE's HAM clock
    gate at 2.4GHz so the real stream starts warm; ~3.4us of PE-busy
    is needed for the 1.2->2.4GHz flip, and any PE idle gap >~1us
    before the flip risks a cold stretch (measured: a 2us gap cost
    ~3us of half-rate matmuls).
  * the r-gate of work item i+1 is computed one iteration early so its
    sigmoid + r*h (ScalarE+VectorE) never gate the candidate matmuls.
  * the last 512 cols split into two 256-col pieces on disjoint PSUM
    regions; both use the short combine (u=z-1, m1=z*h precomputed on
    DVE, so only v=u*hh, o=m1-v remain after the tanh) and the last
    piece is emitted z-gate-first + candidate g-split so after the
    very last matmul only tanh(g1)+2 DVE ops+store remain.  Tail
    stores avoid scalar's queue except the final g1 (a 0.65us store
    trigger between the final tanhs measurably delays the drain).
"""

import os
import sys

for _p in ("/opt/trn_rl_repo", "/root/.axon_site/_ro/trn_rl_repo"):
    if os.path.isdir(_p) and _p not in sys.path:
        sys.path.append(_p)

import numpy as np

B = 65536
D = 256
N_CORES = 8
S = B // N_CORES  # batch rows per core
CH = 512  # batch columns per PSUM bank / compute sub-chunk

# Input-tile load plan: (col_start, width).  The first two are per-block
# (pipeline head fill); the rest are packed 4-block loads.
PLAN = [(0, 512), (512, 512), (1024, 512), (1536, 512)] + [
    (2048 + 1024 * i, 1024) for i in range(6)
]
_BLOCKS = ("x0", "x1", "h0", "h1")
_WORDER = ("Wr", "Ur", "Wz", "Uz", "Wh", "Uh")

# Work items: (dram col start, width, psum col offset).
WI = [(i * CH, CH, 0) for i in range(15)] + [
    (7680, 256, 0),
    (7936, 256, 256),
]


def build_nc(s=S, mm_dtype_name=None):
    """Build + compile the per-core Bass program for a shard of s rows."""
    import concourse.bass as bass
    import concourse.mybir as mybir
    import concourse.tile as tile
    from concourse import bacc

    f32 = mybir.dt.float32
    if mm_dtype_name is None:
        mm_dtype_name = os.environ.get("GRU_MM_DTYPE", "float16")
    f16 = getattr(mybir.dt, mm_dtype_name)
    AF = mybir.ActivationFunctionType
    n_warm = int(os.environ.get("GRU_WARMUP", "10"))

    nc = bacc.Bacc("TRN2", target_bir_lowering=False)
    xh = nc.dram_tensor("xh", [128, 4, s], f16, kind="ExternalInput")
    wcat = nc.dram_tensor("wcat", [D, 6 * D], f16, kind="ExternalInput")
    bcat = nc.dram_tensor("bcat", [128, 6], f32, kind="ExternalInput")
    outT = nc.dram_tensor("outT", [D, s], f16, kind="ExternalOutput")

    nwi = len(WI)

    with tile.TileContext(nc) as tc:
        with (
            tc.tile_pool(name="const", bufs=1) as cpool,
            tc.tile_pool(name="work", bufs=2) as wpool,
            tc.tile_pool(name="outb", bufs=4) as opool,
            tc.tile_pool(name="psum", bufs=1, space=bass.MemorySpace.PSUM) as ppool,
        ):
            inp = {}  # (block, load_idx) -> AP [128, width]

            # PE warm-up: the HAM clock gate needs ~3.4us of sustained PE
            # activity to lift the engine from 1.2 to 2.4 GHz.  The PE is
            # idle during the head DMAs anyway, so burn that window on
            # dummy matmuls over a memset tile.
            zt0 = cpool.tile([128, CH], f16, tag="warm", name="warm")
            nc.gpsimd.memset(zt0[:], 0)
            pw = ppool.tile([128, CH], f32, tag="pwarm", name="pwarm")
            for _ in range(n_warm):
                nc.tensor.matmul(pw[:], zt0[:, 0:128], zt0[:], start=True, stop=True)
            wsink = cpool.tile([128, CH], f32, tag="wsink", name="wsink")
            nc.vector.tensor_copy(wsink[:], pw[:])

            # Head DMA scheduling: HWDGE rings are FIFO per issuing engine
            # with ~8 completion-sem lanes; a trigger past that blocks its
            # engine queue until an earlier DMA completes.  The first slots
            # carry exactly the critical set (r-gate weights + j=0 input
            # tiles); h tiles and stragglers ride GpSimd's SWDGE ring; the
            # bulk stream self-paces on sync's lane rotation.
            wA, wB = {}, {}
            for k in range(2):
                wA[k] = cpool.tile([128, 2 * D], f16, tag=f"wA{k}", name=f"wA{k}")
                wB[k] = cpool.tile([128, 4 * D], f16, tag=f"wB{k}", name=f"wB{k}")

            def load_block(blk, li, eng):
                bi = _BLOCKS.index(blk)
                start, width = PLAN[li]
                t = cpool.tile([128, width], f16, tag=f"i{blk}_{li}",
                               name=f"i{blk}_{li}")
                eng.dma_start(t[:], xh[:, bi, start : start + width])
                inp[(blk, li)] = t[:]

            # wave 1: r-gate weights and the j=0 input tiles - exactly what
            # the first matmuls consume - plus the tiny bias vector
            nc.sync.dma_start(wA[0][:], wcat[0:128, 0 : 2 * D])
            nc.scalar.dma_start(wA[1][:], wcat[128:256, 0 : 2 * D])
            load_block("x0", 0, nc.sync)
            load_block("x1", 0, nc.scalar)
            load_block("h0", 0, nc.gpsimd)
            load_block("h1", 0, nc.gpsimd)
            b_sb = cpool.tile([128, 6], f32, tag="bcat")
            nc.sync.dma_start(b_sb[:], bcat[:, :])
            nc.scalar.dma_start(wB[0][:], wcat[0:128, 2 * D : 6 * D])
            nc.scalar.dma_start(wB[1][:], wcat[128:256, 2 * D : 6 * D])
            # force both ACT tables (sigmoid + tanh) to load now, right
            # after scalar's head triggers: lazily the tanh table load
            # would sit mid-queue and stall the first r-sigmoid
            dume = cpool.tile([128, 1], f16, tag="dume", name="dume")
            nc.scalar.activation(dume[:], zt0[:, 0:1], AF.Sigmoid)
            nc.scalar.activation(dume[:], zt0[:, 0:1], AF.Tanh)
            # rotation-paced stragglers
            load_block("x0", 1, nc.sync)
            load_block("h0", 1, nc.sync)
            load_block("x1", 1, nc.gpsimd)
            load_block("h1", 1, nc.gpsimd)
            for li in range(2, len(PLAN)):
                start, width = PLAN[li]
                t = cpool.tile([128, 4, width], f16, tag=f"ixh_{li}",
                               name=f"ixh_{li}")
                nc.sync.dma_start(t[:], xh[:, :, start : start + width])
                for bi, blk in enumerate(_BLOCKS):
                    inp[(blk, li)] = t[:, bi, :]

            def wap(i, k, g):
                """Weight AP [128,128] for matrix index i (order _WORDER),
                contraction half k, output-feature half g."""
                if i < 2:
                    return wA[k][:, i * D + g * 128 : i * D + (g + 1) * 128]
                return wB[k][:, (i - 2) * D + g * 128 : (i - 2) * D + (g + 1) * 128]

            def inp_ap(blk, c0, w):
                for li, (start, width) in enumerate(PLAN):
                    if start <= c0 and c0 + w <= start + width:
                        return inp[(blk, li)][:, c0 - start : c0 - start + w]
                raise ValueError((blk, c0, w))

            def operands(i):
                c0, w, _ = WI[i]
                xs = [inp_ap(f"x{k}", c0, w) for k in range(2)]
                hs = [inp_ap(f"h{k}", c0, w) for k in range(2)]
                return xs, hs

            def gate_pair(tagbase, wi, ui, xs, rhs_u, po, w):
                """Both g-halves of one gate.  W (x-side) matmuls of both
                halves run before the U matmuls; k-major within each pass."""
                ps = []
                for g in range(2):
                    p = ppool.tile([128, CH], f32, tag=f"{tagbase}{g}",
                                   name=f"{tagbase}{g}")
                    ps.append(p)
                for k in range(2):
                    for g in range(2):
                        nc.tensor.matmul(ps[g][:, po : po + w], wap(wi, k, g),
                                         xs[k], start=(k == 0), stop=False)
                for k in range(2):
                    for g in range(2):
                        nc.tensor.matmul(ps[g][:, po : po + w], wap(ui, k, g),
                                         rhs_u[k], start=False, stop=(k == 1))
                return ps

            def r_gate(i):
                """reset gate -> r*h tiles for work item i."""
                c0, w, po = WI[i]
                xs, hs = operands(i)
                ps = gate_pair("pr", 0, 1, xs, hs, po, w)
                rh = []
                for g in range(2):
                    rt = wpool.tile([128, CH], f16, tag=f"r{g}", name=f"r{g}")
                    nc.scalar.activation(rt[:, 0:w], ps[g][:, po : po + w],
                                         AF.Sigmoid, bias=b_sb[:, g : g + 1])
                    t = wpool.tile([128, CH], f16, tag=f"rh{g}", name=f"rh{g}")
                    nc.vector.tensor_mul(t[:, 0:w], rt[:, 0:w], hs[g])
                    rh.append(t[:, 0:w])
                return rh

            def zu_gate(i, emit_um):
                """z-sigmoids for item i; with emit_um also u=z-1, m1=z*h
                (on DVE, off the critical tail path) so only two DVE links
                remain after the final tanh: o = z*h+(1-z)*hh = m1-u*hh."""
                c0, w, po = WI[i]
                xs, hs = operands(i)
                pz = gate_pair("pz", 2, 3, xs, hs, po, w)
                zt, ut, m1 = [], [], []
                for g in range(2):
                    t = wpool.tile([128, CH], f16, tag=f"z{g}", name=f"z{g}")
                    nc.scalar.activation(t[:, 0:w], pz[g][:, po : po + w],
                                         AF.Sigmoid, bias=b_sb[:, 2 + g : 3 + g])
                    zt.append(t)
                    if emit_um:
                        u = wpool.tile([128, CH], f16, tag=f"u{g}", name=f"u{g}")
                        nc.vector.tensor_scalar_sub(u[:, 0:w], t[:, 0:w], 1.0)
                        ut.append(u)
                        m = wpool.tile([128, CH], f16, tag=f"zh{g}", name=f"zh{g}")
                        nc.vector.tensor_mul(m[:, 0:w], t[:, 0:w], hs[g])
                        m1.append(m)
                return zt, ut, m1

            def combine_short(g, ut, m1, hh, w):
                v = wpool.tile([128, CH], f16, tag=f"v{g}", name=f"v{g}")
                nc.vector.tensor_mul(v[:, 0:w], ut[g][:, 0:w], hh[:, 0:w])
                o = opool.tile([128, CH], f16, tag=f"o{g}", name=f"o{g}")
                nc.vector.tensor_sub(o[:, 0:w], m1[g][:, 0:w], v[:, 0:w])
                return o

            # software pipeline: r-gate one work item ahead of z/candidate.
            rh_cur = r_gate(0)
            for i in range(nwi - 1):
                c0, w, po = WI[i]
                xs, hs = operands(i)
                if i == 0:
                    rh_next = None
                elif i == 1:
                    rh_cur = r_gate(1)
                    rh_next = r_gate(2) if nwi > 2 else None
                else:
                    rh_next = r_gate(i + 1) if i + 1 < nwi else None

                tail = i == nwi - 2
                zt, ut, m1 = zu_gate(i, tail)
                ph = gate_pair("ph", 4, 5, xs, rh_cur, po, w)
                for g in range(2):
                    hh = wpool.tile([128, CH], f16, tag=f"hh{g}", name=f"hh{g}")
                    nc.scalar.activation(hh[:, 0:w], ph[g][:, po : po + w],
                                         AF.Tanh, bias=b_sb[:, 4 + g : 5 + g])
                    if tail:
                        o = combine_short(g, ut, m1, hh, w)
                    else:
                        o = opool.tile([128, CH], f16, tag=f"o{g}", name=f"o{g}")
                        d = wpool.tile([128, CH], f16, tag=f"d{g}", name=f"d{g}")
                        nc.vector.tensor_sub(d[:, 0:w], hs[g], hh[:, 0:w])
                        m = wpool.tile([128, CH], f16, tag=f"m{g}", name=f"m{g}")
                        nc.vector.tensor_mul(m[:, 0:w], zt[g][:, 0:w], d[:, 0:w])
                        nc.vector.tensor_add(o[:, 0:w], hh[:, 0:w], m[:, 0:w])
                    orow = outT[g * 128 : (g + 1) * 128, :]
                    # bulk stores ride gpsimd's SWDGE ring; the second-to-
                    # last item's stores go on sync (NOT scalar: a store
                    # trigger between the final tanhs delays the drain).
                    eng = nc.sync if tail else nc.gpsimd
                    eng.dma_start(orow[:, c0 : c0 + w], o[:, 0:w])
                rh_cur = rh_next

            # Last item, drain-optimized: z first (sigmoid + u/m1 while the
            # candidate matmuls run), candidate g-split so after the very
            # last matmul only tanh(g1) -> v -> o -> store remain.
            i = nwi - 1
            c0, w, po = WI[i]
            xs, hs = operands(i)
            zt, ut, m1 = zu_gate(i, True)
            for g in range(2):
                p = ppool.tile([128, CH], f32, tag=f"ph{g}", name=f"ph{g}")
                for k in range(2):
                    nc.tensor.matmul(p[:, po : po + w], wap(4, k, g), xs[k],
                                     start=(k == 0), stop=False)
                for k in range(2):
                    nc.tensor.matmul(p[:, po : po + w], wap(5, k, g),
                                     rh_cur[k], start=False, stop=(k == 1))
                hh = wpool.tile([128, CH], f16, tag=f"hh{g}", name=f"hh{g}")
                nc.scalar.activation(hh[:, 0:w], p[:, po : po + w],
                                     AF.Tanh, bias=b_sb[:, 4 + g : 5 + g])
                o = combine_short(g, ut, m1, hh, w)
                orow = outT[g * 128 : (g + 1) * 128, :]
                eng = nc.scalar if g == 1 else nc.sync
                eng.dma_start(orow[:, c0 : c0 + w], o[:, 0:w])

    nc.compile()
    return nc


_NC_CACHE = {}


def _get_nc():
    key = (S, os.environ.get("GRU_MM_DTYPE", "float16"),
           os.environ.get("GRU_WARMUP", "10"))
    if key not in _NC_CACHE:
        _NC_CACHE[key] = build_nc(S, key[1])
    return _NC_CACHE[key]


def _make_in_maps(inputs):
    f32 = np.float32
    dt16 = {"float16": np.float16}.get(
        os.environ.get("GRU_MM_DTYPE", "float16")
    )
    if dt16 is None:
        import ml_dtypes

        dt16 = ml_dtypes.bfloat16
    x = np.asarray(inputs["x"], f32)
    h = np.asarray(inputs["h_t_1"], f32)
    wcat = np.ascontiguousarray(
        np.concatenate(
            [np.asarray(inputs[n], f32) for n in ("Wr", "Ur", "Wz", "Uz", "Wh", "Uh")],
            axis=1,
        ).astype(dt16)
    )
    bcat = np.ascontiguousarray(
        np.concatenate(
            [np.asarray(inputs[n], f32).reshape(2, 128).T for n in ("br", "bz", "bh")],
            axis=1,
        )
    )
    consts = {"wcat": wcat, "bcat": bcat}
    in_maps = []
    for c in range(N_CORES):
        sl = slice(c * S, (c + 1) * S)
        xT = x[sl].T.astype(dt16)  # [256, S]
        hT = h[sl].T.astype(dt16)
        xhm = np.empty((128, 4, S), dt16)
        xhm[:, 0] = xT[0:128]
        xhm[:, 1] = xT[128:256]
        xhm[:, 2] = hT[0:128]
        xhm[:, 3] = hT[128:256]
        m = {"xh": np.ascontiguousarray(xhm)}
        m.update(consts)
        in_maps.append(m)
    return in_maps


def run(inputs, trace=False):
    """Run on hardware; returns (h_t ndarray, BassKernelResults)."""
    from concourse.bass_utils import run_bass_kernel_spmd

    nc = _get_nc()
    in_maps = _make_in_maps(inputs)
    res = run_bass_kernel_spmd(nc, in_maps, list(range(N_CORES)), trace=trace)
    out = np.empty((B, D), np.float32)
    for c in range(N_CORES):
        out[c * S : (c + 1) * S] = res.results[c]["outT"].T.astype(np.float32)
    return out, res


def kernel(**inputs):
    out, _ = run(inputs, trace=False)
    return (out, out)


# revision 17
# speedup vs baseline: 1.0253x; 1.0138x over previous
"""GRU cell on 8 Trainium2 NeuronCores.

Reference computation (B=65536, D=256):
    z = sigmoid(x@Wz + h@Uz + bz)
    r = sigmoid(x@Wr + h@Ur + br)
    h_hat = tanh(x@Wh + (r*h)@Uh + bh)
    h_t = z*h + (1-z)*h_hat  ; returns (h_t, h_t)

Strategy: data-parallel over the batch dim (8 shards of 8192 rows), all
fp16 on chip (rel_l2 ~1.1e-3 vs the f32 reference; gate is 2e-2).  The
matmul stream runs at the fp16 PE issue floor (24 matmul passes over
8192 batch cols = 196608 PE cycles = 81.9us at 2.4GHz), so the
remaining time is the fixed framework preamble (~7.2us), the initial
HBM fill, and the drain tail + postamble.  Key structure:
  * host packs each shard as [128 partitions, 4 blocks, 8192] fp16
    (blocks = x k0, x k1, h k0, h k1): the contraction dim of all six
    GEMMs is the SBUF partition dim, fp16 halves HBM traffic and
    streams the PE at full rate with fast weight load.
  * all input tiles are SBUF-resident (8.4MB of 24MB); tiles are 512+
    cols so per-partition DMA lines stay at/above the 1KB efficiency
    knee (256-col tiles fragment to 512B lines and halve effective DMA
    bandwidth - measured).  Head DMAs are spread over the sync/scalar
    HWDGE rings (FIFO per ring, ~8 completion-sem lanes total) and
    gpsimd's SWDGE ring in need-order; the bulk stream self-paces on
    sync's lane rotation.
  * both ACT tables (sigmoid+tanh) are force-loaded via dummy
    activations right after scalar's head triggers - lazily, the tanh
    table load (1.28us) sits mid-queue and stalls the first r-sigmoid
    (measured as candidate-matmul stalls).
  * dummy warm-up matmuls during the head DMAs hold the PE's HAM clock
    gate at 2.4GHz so the real stream starts warm; ~3.4us of PE-busy
    is needed for the 1.2->2.4GHz flip, and any PE idle gap >~1us
    before the flip risks a cold stretch (measured: a 2us gap cost
    ~3us of half-rate matmuls).
  * the r-gate of work item i+1 is computed one iteration early so its
    sigmoid + r*h (ScalarE+VectorE) never gate the candidate matmuls.
  * the last 512 cols split into two 256-col pieces on disjoint PSUM
    regions; both use the short combine (u=z-1, m1=z*h precomputed on
    DVE, so only v=u*hh, o=m1-v remain after the tanh) and the last
    piece is emitted z-gate-first + candidate g-split so after the
    very last matmul only tanh(g1)+2 DVE ops+store remain.  Tail
    stores avoid scalar's queue except the final g1 (a 0.65us store
    trigger between the final tanhs measurably delays the drain).
"""

import os
import sys

for _p in ("/opt/trn_rl_repo", "/root/.axon_site/_ro/trn_rl_repo"):
    if os.path.isdir(_p) and _p not in sys.path:
        sys.path.append(_p)

import numpy as np

B = 65536
D = 256
N_CORES = 8
S = B // N_CORES  # batch rows per core
CH = 512  # batch columns per PSUM bank / compute sub-chunk

# Input-tile load plan: (col_start, width).  The first two are per-block
# (pipeline head fill); the rest are packed 4-block loads.
PLAN = [(0, 512), (512, 512), (1024, 512), (1536, 512)] + [
    (2048 + 1024 * i, 1024) for i in range(6)
]
_BLOCKS = ("x0", "x1", "h0", "h1")
_WORDER = ("Wr", "Ur", "Wz", "Uz", "Wh", "Uh")

# Work items: (dram col start, width, psum col offset).
WI = [(i * CH, CH, 0) for i in range(15)] + [
    (7680, 256, 0),
    (7936, 256, 256),
]


def build_nc(s=S, mm_dtype_name=None):
    """Build + compile the per-core Bass program for a shard of s rows."""
    import concourse.bass as bass
    import concourse.mybir as mybir
    import concourse.tile as tile
    from concourse import bacc

    f32 = mybir.dt.float32
    if mm_dtype_name is None:
        mm_dtype_name = os.environ.get("GRU_MM_DTYPE", "float16")
    f16 = getattr(mybir.dt, mm_dtype_name)
    AF = mybir.ActivationFunctionType
    n_warm = int(os.environ.get("GRU_WARMUP", "10"))

    nc = bacc.Bacc("TRN2", target_bir_lowering=False)
    xh = nc.dram_tensor("xh", [128, 4, s], f16, kind="ExternalInput")
    wcat = nc.dram_tensor("wcat", [D, 6 * D], f16, kind="ExternalInput")
    bcat = nc.dram_tensor("bcat", [128, 6], f32, kind="ExternalInput")
    outT = nc.dram_tensor("outT", [D, s], f16, kind="ExternalOutput")

    nwi = len(WI)

    with tile.TileContext(nc) as tc:
        with (
            tc.tile_pool(name="const", bufs=1) as cpool,
            tc.tile_pool(name="work", bufs=2) as wpool,
            tc.tile_pool(name="outb", bufs=4) as opool,
            tc.tile_pool(name="psum", bufs=1, space=bass.MemorySpace.PSUM) as ppool,
        ):
            inp = {}  # (block, load_idx) -> AP [128, width]

            # PE warm-up: the HAM clock gate needs ~3.4us of sustained PE
            # activity to lift the engine from 1.2 to 2.4 GHz.  The PE is
            # idle during the head DMAs anyway, so burn that window on
            # dummy matmuls over a memset tile.
            zt0 = cpool.tile([128, CH], f16, tag="warm", name="warm")
            nc.gpsimd.memset(zt0[:], 0)
            pw = ppool.tile([128, CH], f32, tag="pwarm", name="pwarm")
            for _ in range(n_warm):
                nc.tensor.matmul(pw[:], zt0[:, 0:128], zt0[:], start=True, stop=True)
            wsink = cpool.tile([128, CH], f32, tag="wsink", name="wsink")
            nc.vector.tensor_copy(wsink[:], pw[:])

            # Head DMA scheduling: HWDGE rings are FIFO per issuing engine
            # with ~8 completion-sem lanes; a trigger past that blocks its
            # engine queue until an earlier DMA completes.  The first slots
            # carry exactly the critical set (r-gate weights + j=0 input
            # tiles); h tiles and stragglers ride GpSimd's SWDGE ring; the
            # bulk stream self-paces on sync's lane rotation.
            wA, wZ, wH = {}, {}, {}
            for k in range(2):
                wA[k] = cpool.tile([128, 2 * D], f16, tag=f"wA{k}", name=f"wA{k}")
                wZ[k] = cpool.tile([128, 2 * D], f16, tag=f"wZ{k}", name=f"wZ{k}")
                wH[k] = cpool.tile([128, 2 * D], f16, tag=f"wH{k}", name=f"wH{k}")

            def load_block(blk, li, eng):
                bi = _BLOCKS.index(blk)
                start, width = PLAN[li]
                t = cpool.tile([128, width], f16, tag=f"i{blk}_{li}",
                               name=f"i{blk}_{li}")
                eng.dma_start(t[:], xh[:, bi, start : start + width])
                inp[(blk, li)] = t[:]

            # wave 1: r-gate weights and the j=0 input tiles - exactly what
            # the first matmuls consume - plus the tiny bias vector
            nc.sync.dma_start(wA[0][:], wcat[0:128, 0 : 2 * D])
            nc.scalar.dma_start(wA[1][:], wcat[128:256, 0 : 2 * D])
            load_block("x0", 0, nc.sync)
            load_block("x1", 0, nc.scalar)
            load_block("h0", 0, nc.gpsimd)
            load_block("h1", 0, nc.gpsimd)
            b_sb = cpool.tile([128, 6], f32, tag="bcat")
            nc.sync.dma_start(b_sb[:], bcat[:, :])
            # z/candidate weights split per-gate so the z chunk lands
            # before the z matmuls need it (a single 4D-wide wB chunk
            # finishes ~2us after the z matmuls want its first half)
            nc.scalar.dma_start(wZ[0][:], wcat[0:128, 2 * D : 4 * D])
            nc.sync.dma_start(wZ[1][:], wcat[128:256, 2 * D : 4 * D])
            nc.scalar.dma_start(wH[0][:], wcat[0:128, 4 * D : 6 * D])
            nc.sync.dma_start(wH[1][:], wcat[128:256, 4 * D : 6 * D])
            # force both ACT tables (sigmoid + tanh) to load now, after
            # scalar's immediate triggers but BEFORE the rotation-blocked
            # stragglers (which hold scalar's queue until earlier DMAs
            # complete): lazily the tanh table load lands after them and
            # stalls the first r-sigmoid by ~1.5us
            dume = cpool.tile([128, 1], f16, tag="dume", name="dume")
            nc.scalar.activation(dume[:], zt0[:, 0:1], AF.Sigmoid)
            nc.scalar.activation(dume[:], zt0[:, 0:1], AF.Tanh)
            # rotation-paced stragglers: these throttle themselves on their
            # engine's completion-sem lanes, so they cannot steal HBM
            # bandwidth from the critical wA/wB/x/h pieces above (measured:
            # moving them to gpsimd's free lanes delayed wB[0] by 2.3us)
            load_block("x0", 1, nc.sync)
            load_block("h0", 1, nc.sync)
            load_block("x1", 1, nc.scalar)
            load_block("h1", 1, nc.scalar)
            for li in range(2, len(PLAN)):
                start, width = PLAN[li]
                t = cpool.tile([128, 4, width], f16, tag=f"ixh_{li}",
                               name=f"ixh_{li}")
                nc.sync.dma_start(t[:], xh[:, :, start : start + width])
                for bi, blk in enumerate(_BLOCKS):
                    inp[(blk, li)] = t[:, bi, :]

            def wap(i, k, g):
                """Weight AP [128,128] for matrix index i (order _WORDER),
                contraction half k, output-feature half g."""
                chunk = (wA, wA, wZ, wZ, wH, wH)[i]
                return chunk[k][:, (i % 2) * D + g * 128 : (i % 2) * D + (g + 1) * 128]

            def inp_ap(blk, c0, w):
                for li, (start, width) in enumerate(PLAN):
                    if start <= c0 and c0 + w <= start + width:
                        return inp[(blk, li)][:, c0 - start : c0 - start + w]
                raise ValueError((blk, c0, w))

            def operands(i):
                c0, w, _ = WI[i]
                xs = [inp_ap(f"x{k}", c0, w) for k in range(2)]
                hs = [inp_ap(f"h{k}", c0, w) for k in range(2)]
                return xs, hs

            def gate_pair(tagbase, wi, ui, xs, rhs_u, po, w):
                """Both g-halves of one gate.  W (x-side) matmuls of both
                halves run before the U matmuls; k-major within each pass."""
                ps = []
                for g in range(2):
                    p = ppool.tile([128, CH], f32, tag=f"{tagbase}{g}",
                                   name=f"{tagbase}{g}")
                    ps.append(p)
                for k in range(2):
                    for g in range(2):
                        nc.tensor.matmul(ps[g][:, po : po + w], wap(wi, k, g),
                                         xs[k], start=(k == 0), stop=False)
                for k in range(2):
                    for g in range(2):
                        nc.tensor.matmul(ps[g][:, po : po + w], wap(ui, k, g),
                                         rhs_u[k], start=False, stop=(k == 1))
                return ps

            def r_gate(i):
                """reset gate -> r*h tiles for work item i."""
                c0, w, po = WI[i]
                xs, hs = operands(i)
                ps = gate_pair("pr", 0, 1, xs, hs, po, w)
                rh = []
                for g in range(2):
                    rt = wpool.tile([128, CH], f16, tag=f"r{g}", name=f"r{g}")
                    nc.scalar.activation(rt[:, 0:w], ps[g][:, po : po + w],
                                         AF.Sigmoid, bias=b_sb[:, g : g + 1])
                    t = wpool.tile([128, CH], f16, tag=f"rh{g}", name=f"rh{g}")
                    nc.vector.tensor_mul(t[:, 0:w], rt[:, 0:w], hs[g])
                    rh.append(t[:, 0:w])
                return rh

            def zu_gate(i, emit_um):
                """z-sigmoids for item i; with emit_um also u=z-1, m1=z*h
                (on DVE, off the critical tail path) so only two DVE links
                remain after the final tanh: o = z*h+(1-z)*hh = m1-u*hh."""
                c0, w, po = WI[i]
                xs, hs = operands(i)
                pz = gate_pair("pz", 2, 3, xs, hs, po, w)
                zt, ut, m1 = [], [], []
                for g in range(2):
                    t = wpool.tile([128, CH], f16, tag=f"z{g}", name=f"z{g}")
                    nc.scalar.activation(t[:, 0:w], pz[g][:, po : po + w],
                                         AF.Sigmoid, bias=b_sb[:, 2 + g : 3 + g])
                    zt.append(t)
                    if emit_um:
                        u = wpool.tile([128, CH], f16, tag=f"u{g}", name=f"u{g}")
                        nc.vector.tensor_scalar_sub(u[:, 0:w], t[:, 0:w], 1.0)
                        ut.append(u)
                        m = wpool.tile([128, CH], f16, tag=f"zh{g}", name=f"zh{g}")
                        nc.vector.tensor_mul(m[:, 0:w], t[:, 0:w], hs[g])
                        m1.append(m)
                return zt, ut, m1

            def combine_short(g, ut, m1, hh, w):
                v = wpool.tile([128, CH], f16, tag=f"v{g}", name=f"v{g}")
                nc.vector.tensor_mul(v[:, 0:w], ut[g][:, 0:w], hh[:, 0:w])
                o = opool.tile([128, CH], f16, tag=f"o{g}", name=f"o{g}")
                nc.vector.tensor_sub(o[:, 0:w], m1[g][:, 0:w], v[:, 0:w])
                return o

            # software pipeline: r-gate one work item ahead of z/candidate.
            rh_cur = r_gate(0)
            for i in range(nwi - 1):
                c0, w, po = WI[i]
                xs, hs = operands(i)
                if i == 0:
                    rh_next = None
                elif i == 1:
                    rh_cur = r_gate(1)
                    rh_next = r_gate(2) if nwi > 2 else None
                else:
                    rh_next = r_gate(i + 1) if i + 1 < nwi else None

                tail = i == nwi - 2
                zt, ut, m1 = zu_gate(i, tail)
                ph = gate_pair("ph", 4, 5, xs, rh_cur, po, w)
                for g in range(2):
                    hh = wpool.tile([128, CH], f16, tag=f"hh{g}", name=f"hh{g}")
                    nc.scalar.activation(hh[:, 0:w], ph[g][:, po : po + w],
                                         AF.Tanh, bias=b_sb[:, 4 + g : 5 + g])
                    if tail:
                        o = combine_short(g, ut, m1, hh, w)
                    else:
                        o = opool.tile([128, CH], f16, tag=f"o{g}", name=f"o{g}")
                        d = wpool.tile([128, CH], f16, tag=f"d{g}", name=f"d{g}")
                        nc.vector.tensor_sub(d[:, 0:w], hs[g], hh[:, 0:w])
                        m = wpool.tile([128, CH], f16, tag=f"m{g}", name=f"m{g}")
                        nc.vector.tensor_mul(m[:, 0:w], zt[g][:, 0:w], d[:, 0:w])
                        nc.vector.tensor_add(o[:, 0:w], hh[:, 0:w], m[:, 0:w])
                    orow = outT[g * 128 : (g + 1) * 128, :]
                    # bulk stores ride gpsimd's SWDGE ring; the second-to-
                    # last item's stores go on sync (NOT scalar: a store
                    # trigger between the final tanhs delays the drain).
                    eng = nc.sync if tail else nc.gpsimd
                    eng.dma_start(orow[:, c0 : c0 + w], o[:, 0:w])
                rh_cur = rh_next

            # Last item, drain-optimized: z first (sigmoid + u/m1 while the
            # candidate matmuls run), candidate g-split so after the very
            # last matmul only tanh(g1) -> v -> o -> store remain.
            i = nwi - 1
            c0, w, po = WI[i]
            xs, hs = operands(i)
            zt, ut, m1 = zu_gate(i, True)
            for g in range(2):
                p = ppool.tile([128, CH], f32, tag=f"ph{g}", name=f"ph{g}")
                for k in range(2):
                    nc.tensor.matmul(p[:, po : po + w], wap(4, k, g), xs[k],
                                     start=(k == 0), stop=False)
                for k in range(2):
                    nc.tensor.matmul(p[:, po : po + w], wap(5, k, g),
                                     rh_cur[k], start=False, stop=(k == 1))
                hh = wpool.tile([128, CH], f16, tag=f"hh{g}", name=f"hh{g}")
                nc.scalar.activation(hh[:, 0:w], p[:, po : po + w],
                                     AF.Tanh, bias=b_sb[:, 4 + g : 5 + g])
                o = combine_short(g, ut, m1, hh, w)
                orow = outT[g * 128 : (g + 1) * 128, :]
                eng = nc.scalar if g == 1 else nc.sync
                eng.dma_start(orow[:, c0 : c0 + w], o[:, 0:w])

    nc.compile()
    return nc


_NC_CACHE = {}


def _get_nc():
    key = (S, os.environ.get("GRU_MM_DTYPE", "float16"),
           os.environ.get("GRU_WARMUP", "10"))
    if key not in _NC_CACHE:
        _NC_CACHE[key] = build_nc(S, key[1])
    return _NC_CACHE[key]


def _make_in_maps(inputs):
    f32 = np.float32
    dt16 = {"float16": np.float16}.get(
        os.environ.get("GRU_MM_DTYPE", "float16")
    )
    if dt16 is None:
        import ml_dtypes

        dt16 = ml_dtypes.bfloat16
    x = np.asarray(inputs["x"], f32)
    h = np.asarray(inputs["h_t_1"], f32)
    wcat = np.ascontiguousarray(
        np.concatenate(
            [np.asarray(inputs[n], f32) for n in ("Wr", "Ur", "Wz", "Uz", "Wh", "Uh")],
            axis=1,
        ).astype(dt16)
    )
    bcat = np.ascontiguousarray(
        np.concatenate(
            [np.asarray(inputs[n], f32).reshape(2, 128).T for n in ("br", "bz", "bh")],
            axis=1,
        )
    )
    consts = {"wcat": wcat, "bcat": bcat}
    in_maps = []
    for c in range(N_CORES):
        sl = slice(c * S, (c + 1) * S)
        xT = x[sl].T.astype(dt16)  # [256, S]
        hT = h[sl].T.astype(dt16)
        xhm = np.empty((128, 4, S), dt16)
        xhm[:, 0] = xT[0:128]
        xhm[:, 1] = xT[128:256]
        xhm[:, 2] = hT[0:128]
        xhm[:, 3] = hT[128:256]
        m = {"xh": np.ascontiguousarray(xhm)}
        m.update(consts)
        in_maps.append(m)
    return in_maps


def run(inputs, trace=False):
    """Run on hardware; returns (h_t ndarray, BassKernelResults)."""
    from concourse.bass_utils import run_bass_kernel_spmd

    nc = _get_nc()
    in_maps = _make_in_maps(inputs)
    res = run_bass_kernel_spmd(nc, in_maps, list(range(N_CORES)), trace=trace)
    out = np.empty((B, D), np.float32)
    for c in range(N_CORES):
        out[c * S : (c + 1) * S] = res.results[c]["outT"].T.astype(np.float32)
    return out, res


def kernel(**inputs):
    out, _ = run(inputs, trace=False)
    return (out, out)
